# revision 26
# baseline (speedup 1.0000x reference)
"""Trainium2 Bass kernel: single-head self-attention with residual.

Reference computation (per batch b):
    q = x @ Wq + bq ; k = x @ Wk + bk ; v = x @ Wv + bv
    scores = q @ k^T / sqrt(U) ; attn = softmax(scores, axis=-1)
    out = x + (attn @ v) @ Wp + bp

Shapes: x [B=4, N=4096, U=512], weights [512, 512], biases [512].

Sharding: 8 cores = 4 batches x 2 sequence halves. Core i owns batch
b = i // 2, Q-rows h = i % 2 (2048 rows). Each core receives its
batch's FULL x (host-side replication plays the role of the K/V
all-gather), so there are no on-device collectives and cores are fully
independent.

Device layout choices:
  - All matmuls run in fp8e4 (TRN E4M3, max +-240) with
    perf_mode=DoubleRow: operands carry a [128, 2, *] AP (stationary
    [128, 2, 128], moving [128, 2, 512]) so each instruction contracts
    256 -- half the instruction count of bf16; measured ~216 ns per MM
    at full clock (~the 2x fp8 peak rate).
  - Scale management keeps every fp8 tensor well below the 240
    overflow: weights are scaled by WS=16 on the host (keeps N(0,1/512)
    entries out of the subnormal range), scores come out 256x, folded
    into the softmax exp scale; exp gets a -SHIFT bias (softmax is
    shift-invariant) so ex <= e^(smax-SHIFT) ~ 45; ctx is written at
    CTX_SCL=1/32. The denominator 'ones' stationary is 8.0 so the
    reciprocal directly absorbs the leftover 16*16*CTX_SCL/256 = 1/8.
  - x arrives pre-transposed from the host (xkvT [U, N]) so K^T / Q^T
    land directly in the [U, seq] layout the TensorEngine wants; no
    on-device transposes at all.
  - Scores are computed transposed (S^T [nk, nq] tiles), exp'd on the
    Scalar engine straight out of PSUM, and consumed as the moving
    operand of the PV matmul -- flash-attention style. The softmax
    denominator is a DoubleRow ones-matmul over the ex tiles.
  - Score PSUM tiles are PAIRED ([128, 2, 512] spanning 2 banks, one
    exp per pair) and the output projection shares the ctx PSUM ring,
    whose natural slot rotation (ctx0-3 -> pj0-3 -> next ctx0-3)
    pipelines the per-tile tail without explicit deferral.
  - HBM pressure: phase B streams only bf16 xq (prefetched once) and
    bf16 outputs; profiling showed fp32 residual/output traffic pushed
    HBM utilization to ~0.9 and co-limited the kernel.
  - bv/bp are folded on the host: attn rows sum to 1, so
    out = xq + (attn @ (x@Wv)) @ Wp + (bv @ Wp + bp).
"""

import numpy as np

B, N, U = 4, 4096, 512
NCORES = 8
NQ = N // 2          # 2048 Q rows per core
P = 128              # partitions
C = U // P           # 4 u-chunks
NKC = N // P         # 32 nk chunks
NKT = N // 512       # 8 nk 512-tiles
NQT = NQ // 512      # 4 nq 512-tiles
NQC = NQ // P        # 16 q-row chunks of 128
SCALE = float(1.0 / np.sqrt(np.float32(U)))
WS = 16.0            # host-side weight scale (keeps W out of fp8 subnormals)
SHIFT = 3.0          # softmax shift: ex = exp(s - SHIFT)
CTX_SCL = 1.0 / 32.0  # ctx psum -> fp8 scale
# exp input: psum = (16q).(16k) = 256*qk  ->  scale = SCALE/256
SCALE_EFF = SCALE / (WS * WS)
# out = pj * recip + xq needs recip = 1/(WS*WS*CTX_SCL*den) = 1/(8*den):
# the denominator 'ones' stationary is DEN_W so reciprocal(DEN_W*den) works.
DEN_W = WS * WS * CTX_SCL

_CACHE = {}


def warm_ps_out(t):
    return t[:]


def _build_nc():
    from concourse import bacc, mybir, tile

    f32 = mybir.dt.float32
    bf16 = mybir.dt.bfloat16
    f8 = mybir.dt.float8e4
    Ident = mybir.ActivationFunctionType.Identity
    Exp = mybir.ActivationFunctionType.Exp
    Mult = mybir.AluOpType.mult
    Add = mybir.AluOpType.add
    DR = mybir.MatmulPerfMode.DoubleRow

    nc = bacc.Bacc("TRN2", target_bir_lowering=False, debug=False, num_devices=NCORES)

    xkvT_d = nc.dram_tensor("xkvT", [U, NQ], f8, kind="ExternalInput")
    xqT_d = nc.dram_tensor("xqT", [U, NQ], f8, kind="ExternalInput")
    xq_d = nc.dram_tensor("xq", [NQ, U], bf16, kind="ExternalInput")
    Wq_d = nc.dram_tensor("Wq", [U, U], f8, kind="ExternalInput")
    Wk_d = nc.dram_tensor("Wk", [U, U], f8, kind="ExternalInput")
    Wv_d = nc.dram_tensor("Wv", [U, U], f8, kind="ExternalInput")
    Wp_d = nc.dram_tensor("Wp", [U, U], f8, kind="ExternalInput")
    bq_d = nc.dram_tensor("bq", [U], f32, kind="ExternalInput")
    bk_d = nc.dram_tensor("bk", [U], f32, kind="ExternalInput")
    out_d = nc.dram_tensor("out", [NQ, U], bf16, kind="ExternalOutput")

    with tile.TileContext(nc) as tc:
        with (
            tc.tile_pool(name="big", bufs=1) as big,
            tc.tile_pool(name="small", bufs=1) as small,
            tc.tile_pool(name="dram", bufs=2, space="DRAM") as dramp,
        ):
            # ---- persistent tensors -------------------------------------
            kT = big.tile([P, C, N], f8, tag="kT")        # 16*k^T  [u, nk]
            qT = big.tile([P, C, NQ], f8, tag="qT")       # 16*q^T  [u, nq]
            v = big.tile([P, NKC, U], f8, tag="v")        # 16*v    [nk, u]
            xq_sb = big.tile([P, NQC, U], bf16, tag="xq_sb")  # residual+bconst

            bq_sb = small.tile([P, C], f32, tag="bq")
            bk_sb = small.tile([P, C], f32, tag="bk")
            nc.sync.dma_start(bq_sb[:], bq_d.ap().rearrange("(c p) -> p c", p=P))
            nc.sync.dma_start(bk_sb[:], bk_d.ap().rearrange("(c p) -> p c", p=P))

            ones2 = small.tile([P, 2, 16], f8, tag="ones2")
            nc.vector.memset(ones2[:], DEN_W)
            negshift = small.tile([P, 1], f32, tag="negshift")
            nc.vector.memset(negshift[:], -SHIFT)
            one_one = small.tile([1, 1], f32, tag="one_one")
            nc.vector.memset(one_one[:], 1.0)

            xkvT_r = xkvT_d.ap().rearrange("(c p) n -> p c n", p=P)
            xqT_r = xqT_d.ap().rearrange("(c p) n -> p c n", p=P)

            # ---- phase A: projections -----------------------------------
            with (
                tc.tile_pool(name="w3", bufs=1) as w3,
                tc.tile_pool(name="stream", bufs=4) as stream,
                tc.tile_pool(name="pa_ps", bufs=6, space="PSUM") as pa_ps,
            ):
                warm = w3.tile([P, 512], bf16, tag="warm")
                nc.vector.memset(warm[:], 0.0)
                wps = pa_ps.tile([P, 512], f32, tag="warm_ps", name="warm_ps",
                                 bufs=1)
                for i in range(20):
                    nc.tensor.matmul(
                        warm_ps_out(wps), warm[:, 0:P], warm[:],
                        start=(i == 0), stop=False,
                    )

                wq = w3.tile([P, C, U], f8, tag="wq")
                wk = w3.tile([P, C, U], f8, tag="wk")
                wv = w3.tile([P, C, U], f8, tag="wv")
                wq_r = Wq_d.ap().rearrange("(c p) n -> p c n", p=P)
                wk_r = Wk_d.ap().rearrange("(c p) n -> p c n", p=P)
                wv_r = Wv_d.ap().rearrange("(c p) n -> p c n", p=P)
                for c in range(C):
                    nc.sync.dma_start(wq[:, c, :], wq_r[:, c, :])

                # qT[u_out, i] = sum_u_in Wq[u_in, u_out] * xqT[u_in, i] (+bq)
                xts_q = []
                for t in range(NQT):
                    xt = stream.tile([P, C, 512], f8, tag="xt",
                                     name=f"xt_q{t}")
                    xts_q.append(xt)
                    for c in range(C):
                        nc.sync.dma_start(
                            xt[:, c, :], xqT_r[:, c, t * 512:(t + 1) * 512])
                    if t == 2:
                        for c in range(C):
                            nc.sync.dma_start(wk[:, c, :], wk_r[:, c, :])
                            nc.sync.dma_start(wv[:, c, :], wv_r[:, c, :])
                    for m in range(C):
                        ps = pa_ps.tile([P, 512], f32, tag="pa")
                        for c2 in range(2):
                            nc.tensor.matmul(
                                ps[:], wq[:, 2 * c2:2 * c2 + 2, m * P:(m + 1) * P],
                                xt[:, 2 * c2:2 * c2 + 2, :],
                                start=(c2 == 0), stop=(c2 == 1),
                                perf_mode=DR,
                            )
                        if m % 2 == 0:
                            nc.scalar.activation(
                                qT[:, m, t * 512:(t + 1) * 512], ps[:], Ident,
                                bias=bq_sb[:, m:m + 1],
                            )
                        else:
                            nc.vector.tensor_scalar_add(
                                qT[:, m, t * 512:(t + 1) * 512], ps[:],
                                bq_sb[:, m:m + 1],
                            )
                    if t in (1, 2):  # keep the PE hot through the DMA wait
                        for i in range(12 if t == 1 else 20):
                            nc.tensor.matmul(
                                warm_ps_out(wps), warm[:, 0:P], warm[:],
                                start=False, stop=False,
                            )

                # kT like qT; v[j, u] = sum_u_in x^T[u_in, j] * Wv[u_in, u]
                # key tiles 0-3 are the (resident) Q tiles: keys are laid
                # out [own half, other half] -- a permutation softmax is
                # invariant to since kT and v share it. Tiles 4-7 stream
                # the sibling half from xkvT.
                for t0 in range(0, NKT, 2):
                    if t0 < NQT:
                        xts = [xts_q[t0], xts_q[t0 + 1]]
                    else:
                        xts = []
                        for t in (t0, t0 + 1):
                            xt = stream.tile([P, C, 512], f8, tag="xt",
                                             name=f"xt_kv_{t}")
                            for c in range(C):
                                nc.sync.dma_start(
                                    xt[:, c, :],
                                    xkvT_r[:, c, (t - NQT) * 512:
                                           (t - NQT + 1) * 512])
                            xts.append(xt)
                    for t, xt in zip((t0, t0 + 1), xts):
                        for m in range(C):
                            ps = pa_ps.tile([P, 512], f32, tag="pa")
                            for c2 in range(2):
                                nc.tensor.matmul(
                                    ps[:],
                                    wk[:, 2 * c2:2 * c2 + 2, m * P:(m + 1) * P],
                                    xt[:, 2 * c2:2 * c2 + 2, :],
                                    start=(c2 == 0), stop=(c2 == 1),
                                    perf_mode=DR,
                                )
                            nc.scalar.activation(
                                kT[:, m, t * 512:(t + 1) * 512], ps[:], Ident,
                                bias=bk_sb[:, m:m + 1],
                            )
                    for t, xt in zip((t0, t0 + 1), xts):
                        for m in range(4):  # nk sub-chunks of this 512-tile
                            ps = pa_ps.tile([P, 512], f32, tag="pa")
                            for c2 in range(2):
                                nc.tensor.matmul(
                                    ps[:],
                                    xt[:, 2 * c2:2 * c2 + 2, m * P:(m + 1) * P],
                                    wv[:, 2 * c2:2 * c2 + 2, :],
                                    start=(c2 == 0), stop=(c2 == 1),
                                    perf_mode=DR,
                                )
                            nc.vector.tensor_copy(v[:, t * 4 + m, :], ps[:])

                # bridge the pool-scope transition so the PE clock stays hot
                for i in range(8):
                    nc.tensor.matmul(
                        warm_ps_out(wps), warm[:, 0:P], warm[:],
                        start=False, stop=(i == 7),
                    )
                nc.scalar.copy(warm[:, 0:4], wps[:, 0:4])  # retire warm psum

            # ---- phase B: attention + projection ------------------------
            with (
                tc.tile_pool(name="wpp", bufs=1) as wpp,
                tc.tile_pool(name="expp", bufs=2) as expp,
                tc.tile_pool(name="ctxp", bufs=2) as ctxp,
                tc.tile_pool(name="io", bufs=3) as iop,
                tc.tile_pool(name="st_ps", bufs=2, space="PSUM") as st_ps,
                tc.tile_pool(name="ctx_ps", bufs=4, space="PSUM") as ctx_ps,
            ):
                wp = wpp.tile([P, C, U], f8, tag="wp")
                nc.sync.dma_start(wp[:], Wp_d.ap().rearrange("(c p) n -> p c n", p=P))
                # residual (+ folded bias) prefetch, bf16, one strided DMA
                nc.sync.dma_start(
                    xq_sb[:], xq_d.ap().rearrange("(c p) u -> p c u", p=P))

                for t in range(NQT):
                    nq_sl = slice(t * 512, (t + 1) * 512)
                    ctx_psums = [
                        ctx_ps.tile([P, 512], f32, tag="ctx", name=f"ctx_{t}_{u}")
                        for u in range(C)
                    ]

                    ex = expp.tile([P, NKC, 512], f8, tag="ex")
                    for kk in range(0, NKC, 4):
                        for nk in range(kk, kk + 4, 2):
                            # paired score tiles: one 2-bank PSUM tile, one
                            # [128, 2, 512] exp per two key chunks
                            st = st_ps.tile([P, 2, 512], f32, tag="st")
                            for j in range(2):
                                for c2 in range(2):
                                    nc.tensor.matmul(
                                        st[:, j, :],
                                        kT[:, 2 * c2:2 * c2 + 2,
                                           (nk + j) * P:(nk + j + 1) * P],
                                        qT[:, 2 * c2:2 * c2 + 2, nq_sl],
                                        start=(c2 == 0), stop=(c2 == 1),
                                        perf_mode=DR,
                                    )
                            nc.scalar.activation(
                                ex[:, nk:nk + 2, :], st[:], Exp,
                                scale=SCALE_EFF, bias=negshift[:])
                        for k2 in range(kk // 2, kk // 2 + 2):
                            for u in range(C):
                                nc.tensor.matmul(
                                    ctx_psums[u][:],
                                    v[:, 2 * k2:2 * k2 + 2, u * P:(u + 1) * P],
                                    ex[:, 2 * k2:2 * k2 + 2, :],
                                    start=(k2 == 0), stop=(k2 == NKC // 2 - 1),
                                    perf_mode=DR,
                                )
                    # softmax denominator: DoubleRow ones-matmul over ex;
                    # den row lives in bank 0 of an st-ring slot, its
                    # transpose (denT) in bank 1 of the same slot.
                    den = st_ps.tile([P, 2, 512], f32, tag="st", name=f"den_{t}")
                    for k2 in range(NKC // 2):
                        nc.tensor.matmul(
                            den[0:1, 0, :], ones2[:, 0:2, 0:1],
                            ex[:, 2 * k2:2 * k2 + 2, :],
                            start=(k2 == 0), stop=(k2 == NKC // 2 - 1),
                            perf_mode=DR,
                        )
                    ctxT = ctxp.tile([P, C, 512], f8, tag="ctxT")
                    for u in range(C):
                        nc.vector.tensor_scalar_mul(
                            ctxT[:, u, :], ctx_psums[u][:], CTX_SCL)

                    den_row = small.tile([1, 512], f32, tag="den_row", bufs=2)
                    nc.scalar.copy(den_row[:], den[0:1, 0, :])
                    for s in range(4):
                        # K=1 matmul == transpose of a 128-wide row slice
                        nc.tensor.matmul(
                            den[:, 1, s:s + 1], den_row[0:1, s * P:(s + 1) * P],
                            one_one[:], start=True, stop=True,
                        )
                    recip = small.tile([P, C], f32, tag="recip", bufs=2)
                    nc.vector.reciprocal(recip[:], den[:, 1, 0:4])

                    for s in range(4):  # nq sub-chunks of 128
                        pj = ctx_ps.tile([P, 512], f32, tag="ctx",
                                         name=f"pj_{t}_{s}")
                        for c2 in range(2):
                            nc.tensor.matmul(
                                pj[:],
                                ctxT[:, 2 * c2:2 * c2 + 2, s * P:(s + 1) * P],
                                wp[:, 2 * c2:2 * c2 + 2, :],
                                start=(c2 == 0), stop=(c2 == 1),
                                perf_mode=DR,
                            )
                        r0 = t * 4 + s
                        o = iop.tile([P, U], bf16, tag="o")
                        # o = pj * recip (per-partition) + xq
                        nc.vector.scalar_tensor_tensor(
                            o[:], pj[:], recip[:, s:s + 1], xq_sb[:, r0, :],
                            Mult, Add,
                        )
                        nc.sync.dma_start(
                            out_d[r0 * P:(r0 + 1) * P, :], o[:])

    nc.compile()
    return nc


def _get_nc():
    if "nc" not in _CACHE:
        _CACHE["nc"] = _build_nc()
    return _CACHE["nc"]


def make_in_maps(x, Wq, bq, Wk, bk, Wv, bv, Wp, bp):
    x = np.asarray(x, np.float32)
    Wq = np.asarray(Wq, np.float32)
    bq = np.asarray(bq, np.float32)
    Wk = np.asarray(Wk, np.float32)
    bk = np.asarray(bk, np.float32)
    Wv = np.asarray(Wv, np.float32)
    bv = np.asarray(bv, np.float32)
    Wp = np.asarray(Wp, np.float32)
    bp = np.asarray(bp, np.float32)

    # attn rows sum to 1 => bv contributes bv @ Wp to every output row
    bconst = (bv @ Wp + bp).astype(np.float32)

    import ml_dtypes
    f8 = ml_dtypes.float8_e4m3
    bf16 = ml_dtypes.bfloat16

    def q8(a):
        return np.ascontiguousarray(np.clip(a, -240.0, 240.0)).astype(f8)

    Wq8, Wk8 = q8(Wq * WS), q8(Wk * WS)
    Wv8, Wp8 = q8(Wv * WS), q8(Wp * WS)

    in_maps = []
    for core in range(NCORES):
        b, h = core // 2, core % 2
        xb = np.ascontiguousarray(x[b])                       # [N, U]
        xbT8 = q8(xb.T)                                       # [U, N]
        in_maps.append({
            "xkvT": np.ascontiguousarray(xbT8[:, (1 - h) * NQ:(2 - h) * NQ]),
            "xqT": np.ascontiguousarray(xbT8[:, h * NQ:(h + 1) * NQ]),
            "xq": np.ascontiguousarray(
                xb[h * NQ:(h + 1) * NQ] + bconst[None, :]).astype(bf16),
            "Wq": Wq8, "Wk": Wk8, "Wv": Wv8, "Wp": Wp8,
            "bq": bq * WS, "bk": bk * WS,
        })
    return in_maps


def gather_out(results):
    out = np.empty((B, N, U), np.float32)
    for core in range(NCORES):
        b, h = core // 2, core % 2
        out[b, h * NQ:(h + 1) * NQ] = np.asarray(
            results[core]["out"], dtype=np.float32)
    return out


def kernel(x, Wq, bq, Wk, bk, Wv, bv, Wp, bp):
    from concourse.bass_utils import run_bass_kernel_spmd

    nc = _get_nc()
    in_maps = make_in_maps(x, Wq, bq, Wk, bk, Wv, bv, Wp, bp)
    res = run_bass_kernel_spmd(nc, in_maps, core_ids=list(range(NCORES)))
    return gather_out(res.results)


# revision 28
# speedup vs baseline: 1.0052x; 1.0052x over previous
"""Trainium2 Bass kernel: single-head self-attention with residual.

Reference computation (per batch b):
    q = x @ Wq + bq ; k = x @ Wk + bk ; v = x @ Wv + bv
    scores = q @ k^T / sqrt(U) ; attn = softmax(scores, axis=-1)
    out = x + (attn @ v) @ Wp + bp

Shapes: x [B=4, N=4096, U=512], weights [512, 512], biases [512].

Sharding: 8 cores = 4 batches x 2 sequence halves. Core i owns batch
b = i // 2, Q-rows h = i % 2 (2048 rows). Each core receives its
batch's FULL x (host-side replication plays the role of the K/V
all-gather), so there are no on-device collectives and cores are fully
independent.

Device layout choices:
  - All matmuls run in fp8e4 (TRN E4M3, max +-240) with
    perf_mode=DoubleRow: operands carry a [128, 2, *] AP (stationary
    [128, 2, 128], moving [128, 2, 512]) so each instruction contracts
    256 -- half the instruction count of bf16; measured ~216 ns per MM
    at full clock (~the 2x fp8 peak rate).
  - Scale management keeps every fp8 tensor well below the 240
    overflow: weights are scaled by WS=16 on the host (keeps N(0,1/512)
    entries out of the subnormal range), scores come out 256x, folded
    into the softmax exp scale; exp gets a -SHIFT bias (softmax is
    shift-invariant) so ex <= e^(smax-SHIFT) ~ 45; ctx is written at
    CTX_SCL=1/32. The denominator 'ones' stationary is 8.0 so the
    reciprocal directly absorbs the leftover 16*16*CTX_SCL/256 = 1/8.
  - x arrives pre-transposed from the host (xkvT [U, N]) so K^T / Q^T
    land directly in the [U, seq] layout the TensorEngine wants; no
    on-device transposes at all.
  - Scores are computed transposed (S^T [nk, nq] tiles), exp'd on the
    Scalar engine straight out of PSUM, and consumed as the moving
    operand of the PV matmul -- flash-attention style. The softmax
    denominator is a DoubleRow ones-matmul over the ex tiles.
  - Score PSUM tiles are PAIRED ([128, 2, 512] spanning 2 banks, one
    exp per pair) and the output projection shares the ctx PSUM ring,
    whose natural slot rotation (ctx0-3 -> pj0-3 -> next ctx0-3)
    pipelines the per-tile tail without explicit deferral.
  - HBM pressure: phase B streams only bf16 xq (prefetched once) and
    bf16 outputs; profiling showed fp32 residual/output traffic pushed
    HBM utilization to ~0.9 and co-limited the kernel.
  - bv/bp are folded on the host: attn rows sum to 1, so
    out = xq + (attn @ (x@Wv)) @ Wp + (bv @ Wp + bp).
"""

import numpy as np

B, N, U = 4, 4096, 512
NCORES = 8
NQ = N // 2          # 2048 Q rows per core
P = 128              # partitions
C = U // P           # 4 u-chunks
NKC = N // P         # 32 nk chunks
NKT = N // 512       # 8 nk 512-tiles
NQT = NQ // 512      # 4 nq 512-tiles
NQC = NQ // P        # 16 q-row chunks of 128
SCALE = float(1.0 / np.sqrt(np.float32(U)))
WS = 16.0            # host-side weight scale (keeps W out of fp8 subnormals)
SHIFT = 3.0          # softmax shift: ex = exp(s - SHIFT)
CTX_SCL = 1.0 / 32.0  # ctx psum -> fp8 scale
# exp input: psum = (16q).(16k) = 256*qk  ->  scale = SCALE/256
SCALE_EFF = SCALE / (WS * WS)
# out = pj * recip + xq needs recip = 1/(WS*WS*CTX_SCL*den) = 1/(8*den):
# the denominator 'ones' stationary is DEN_W so reciprocal(DEN_W*den) works.
DEN_W = WS * WS * CTX_SCL

_CACHE = {}


def warm_ps_out(t):
    return t[:]


def _build_nc():
    from concourse import bacc, mybir, tile

    f32 = mybir.dt.float32
    bf16 = mybir.dt.bfloat16
    f8 = mybir.dt.float8e4
    Ident = mybir.ActivationFunctionType.Identity
    Exp = mybir.ActivationFunctionType.Exp
    Mult = mybir.AluOpType.mult
    Add = mybir.AluOpType.add
    DR = mybir.MatmulPerfMode.DoubleRow

    nc = bacc.Bacc("TRN2", target_bir_lowering=False, debug=False, num_devices=NCORES)

    xkvT_d = nc.dram_tensor("xkvT", [U, NQ], f8, kind="ExternalInput")
    xqT_d = nc.dram_tensor("xqT", [U, NQ], f8, kind="ExternalInput")
    xq_d = nc.dram_tensor("xq", [NQ, U], bf16, kind="ExternalInput")
    Wq_d = nc.dram_tensor("Wq", [U, U], f8, kind="ExternalInput")
    Wk_d = nc.dram_tensor("Wk", [U, U], f8, kind="ExternalInput")
    Wv_d = nc.dram_tensor("Wv", [U, U], f8, kind="ExternalInput")
    Wp_d = nc.dram_tensor("Wp", [U, U], f8, kind="ExternalInput")
    bq_d = nc.dram_tensor("bq", [U], f32, kind="ExternalInput")
    bk_d = nc.dram_tensor("bk", [U], f32, kind="ExternalInput")
    out_d = nc.dram_tensor("out", [NQ, U], bf16, kind="ExternalOutput")

    with tile.TileContext(nc) as tc:
        with (
            tc.tile_pool(name="big", bufs=1) as big,
            tc.tile_pool(name="small", bufs=1) as small,
            tc.tile_pool(name="dram", bufs=2, space="DRAM") as dramp,
        ):
            # ---- persistent tensors -------------------------------------
            kT = big.tile([P, C, N], f8, tag="kT")        # 16*k^T  [u, nk]
            qT = big.tile([P, C, NQ], f8, tag="qT")       # 16*q^T  [u, nq]
            v = big.tile([P, NKC, U], f8, tag="v")        # 16*v    [nk, u]
            xq_sb = big.tile([P, NQC, U], bf16, tag="xq_sb")  # residual+bconst

            bq_sb = small.tile([P, C], f32, tag="bq")
            bk_sb = small.tile([P, C], f32, tag="bk")
            nc.sync.dma_start(bq_sb[:], bq_d.ap().rearrange("(c p) -> p c", p=P))
            nc.sync.dma_start(bk_sb[:], bk_d.ap().rearrange("(c p) -> p c", p=P))

            ones2 = small.tile([P, 2, 16], f8, tag="ones2")
            nc.vector.memset(ones2[:], DEN_W)
            negshift = small.tile([P, 1], f32, tag="negshift")
            nc.vector.memset(negshift[:], -SHIFT)
            one_one = small.tile([1, 1], f32, tag="one_one")
            nc.vector.memset(one_one[:], 1.0)

            xkvT_r = xkvT_d.ap().rearrange("(c p) n -> p c n", p=P)
            xqT_r = xqT_d.ap().rearrange("(c p) n -> p c n", p=P)

            # ---- phase A: projections -----------------------------------
            with (
                tc.tile_pool(name="w3", bufs=1) as w3,
                tc.tile_pool(name="stream", bufs=4) as stream,
                tc.tile_pool(name="pa_ps", bufs=6, space="PSUM") as pa_ps,
            ):
                warm = w3.tile([P, 512], bf16, tag="warm")
                nc.vector.memset(warm[:], 0.0)
                wps = pa_ps.tile([P, 512], f32, tag="warm_ps", name="warm_ps",
                                 bufs=1)
                for i in range(16):
                    nc.tensor.matmul(
                        warm_ps_out(wps), warm[:, 0:P], warm[:],
                        start=(i == 0), stop=False,
                    )

                wq = w3.tile([P, C, U], f8, tag="wq")
                wk = w3.tile([P, C, U], f8, tag="wk")
                wv = w3.tile([P, C, U], f8, tag="wv")
                wq_r = Wq_d.ap().rearrange("(c p) n -> p c n", p=P)
                wk_r = Wk_d.ap().rearrange("(c p) n -> p c n", p=P)
                wv_r = Wv_d.ap().rearrange("(c p) n -> p c n", p=P)
                for c in range(C):
                    nc.sync.dma_start(wq[:, c, :], wq_r[:, c, :])

                # qT[u_out, i] = sum_u_in Wq[u_in, u_out] * xqT[u_in, i] (+bq)
                xts_q = []
                for t in range(NQT):
                    xt = stream.tile([P, C, 512], f8, tag="xt",
                                     name=f"xt_q{t}")
                    xts_q.append(xt)
                    for c in range(C):
                        nc.sync.dma_start(
                            xt[:, c, :], xqT_r[:, c, t * 512:(t + 1) * 512])
                    if t == 2:
                        for c in range(C):
                            nc.sync.dma_start(wk[:, c, :], wk_r[:, c, :])
                            nc.sync.dma_start(wv[:, c, :], wv_r[:, c, :])
                    for m in range(C):
                        ps = pa_ps.tile([P, 512], f32, tag="pa")
                        for c2 in range(2):
                            nc.tensor.matmul(
                                ps[:], wq[:, 2 * c2:2 * c2 + 2, m * P:(m + 1) * P],
                                xt[:, 2 * c2:2 * c2 + 2, :],
                                start=(c2 == 0), stop=(c2 == 1),
                                perf_mode=DR,
                            )
                        if m % 2 == 0:
                            nc.scalar.activation(
                                qT[:, m, t * 512:(t + 1) * 512], ps[:], Ident,
                                bias=bq_sb[:, m:m + 1],
                            )
                        else:
                            nc.vector.tensor_scalar_add(
                                qT[:, m, t * 512:(t + 1) * 512], ps[:],
                                bq_sb[:, m:m + 1],
                            )
                    if t in (1, 2):  # keep the PE hot through the DMA wait
                        nfill = 12 if t == 1 else 22
                        for i in range(nfill):
                            nc.tensor.matmul(
                                warm_ps_out(wps), warm[:, 0:P], warm[:],
                                start=False, stop=(t == 2 and i == nfill - 1),
                            )
                nc.scalar.copy(warm[:, 0:4], wps[:, 0:4])  # retire warm psum

                # kT like qT; v[j, u] = sum_u_in x^T[u_in, j] * Wv[u_in, u]
                # key tiles 0-3 are the (resident) Q tiles: keys are laid
                # out [own half, other half] -- a permutation softmax is
                # invariant to since kT and v share it. Tiles 4-7 stream
                # the sibling half from xkvT.
                for t0 in range(0, NKT, 2):
                    if t0 < NQT:
                        xts = [xts_q[t0], xts_q[t0 + 1]]
                    else:
                        xts = []
                        for t in (t0, t0 + 1):
                            xt = stream.tile([P, C, 512], f8, tag="xt",
                                             name=f"xt_kv_{t}")
                            for c in range(C):
                                nc.sync.dma_start(
                                    xt[:, c, :],
                                    xkvT_r[:, c, (t - NQT) * 512:
                                           (t - NQT + 1) * 512])
                            xts.append(xt)
                    for t, xt in zip((t0, t0 + 1), xts):
                        for m in range(C):
                            ps = pa_ps.tile([P, 512], f32, tag="pa")
                            for c2 in range(2):
                                nc.tensor.matmul(
                                    ps[:],
                                    wk[:, 2 * c2:2 * c2 + 2, m * P:(m + 1) * P],
                                    xt[:, 2 * c2:2 * c2 + 2, :],
                                    start=(c2 == 0), stop=(c2 == 1),
                                    perf_mode=DR,
                                )
                            nc.scalar.activation(
                                kT[:, m, t * 512:(t + 1) * 512], ps[:], Ident,
                                bias=bk_sb[:, m:m + 1],
                            )
                    for t, xt in zip((t0, t0 + 1), xts):
                        for m in range(4):  # nk sub-chunks of this 512-tile
                            ps = pa_ps.tile([P, 512], f32, tag="pa")
                            for c2 in range(2):
                                nc.tensor.matmul(
                                    ps[:],
                                    xt[:, 2 * c2:2 * c2 + 2, m * P:(m + 1) * P],
                                    wv[:, 2 * c2:2 * c2 + 2, :],
                                    start=(c2 == 0), stop=(c2 == 1),
                                    perf_mode=DR,
                                )
                            nc.vector.tensor_copy(v[:, t * 4 + m, :], ps[:])

            # ---- phase B: attention + projection ------------------------
            with (
                tc.tile_pool(name="wpp", bufs=1) as wpp,
                tc.tile_pool(name="expp", bufs=2) as expp,
                tc.tile_pool(name="ctxp", bufs=2) as ctxp,
                tc.tile_pool(name="io", bufs=3) as iop,
                tc.tile_pool(name="st_ps", bufs=2, space="PSUM") as st_ps,
                tc.tile_pool(name="ctx_ps", bufs=4, space="PSUM") as ctx_ps,
            ):
                wp = wpp.tile([P, C, U], f8, tag="wp")
                nc.sync.dma_start(wp[:], Wp_d.ap().rearrange("(c p) n -> p c n", p=P))
                # residual (+ folded bias) prefetch, bf16, one strided DMA
                nc.sync.dma_start(
                    xq_sb[:], xq_d.ap().rearrange("(c p) u -> p c u", p=P))

                for t in range(NQT):
                    nq_sl = slice(t * 512, (t + 1) * 512)
                    ctx_psums = [
                        ctx_ps.tile([P, 512], f32, tag="ctx", name=f"ctx_{t}_{u}")
                        for u in range(C)
                    ]

                    ex = expp.tile([P, NKC, 512], f8, tag="ex")
                    for kk in range(0, NKC, 4):
                        for nk in range(kk, kk + 4, 2):
                            # paired score tiles: one 2-bank PSUM tile, one
                            # [128, 2, 512] exp per two key chunks
                            st = st_ps.tile([P, 2, 512], f32, tag="st")
                            for j in range(2):
                                for c2 in range(2):
                                    nc.tensor.matmul(
                                        st[:, j, :],
                                        kT[:, 2 * c2:2 * c2 + 2,
                                           (nk + j) * P:(nk + j + 1) * P],
                                        qT[:, 2 * c2:2 * c2 + 2, nq_sl],
                                        start=(c2 == 0), stop=(c2 == 1),
                                        perf_mode=DR,
                                    )
                            nc.scalar.activation(
                                ex[:, nk:nk + 2, :], st[:], Exp,
                                scale=SCALE_EFF, bias=negshift[:])
                        for k2 in range(kk // 2, kk // 2 + 2):
                            for u in range(C):
                                nc.tensor.matmul(
                                    ctx_psums[u][:],
                                    v[:, 2 * k2:2 * k2 + 2, u * P:(u + 1) * P],
                                    ex[:, 2 * k2:2 * k2 + 2, :],
                                    start=(k2 == 0), stop=(k2 == NKC // 2 - 1),
                                    perf_mode=DR,
                                )
                    # softmax denominator: DoubleRow ones-matmul over ex;
                    # den row lives in bank 0 of an st-ring slot, its
                    # transpose (denT) in bank 1 of the same slot.
                    den = st_ps.tile([P, 2, 512], f32, tag="st", name=f"den_{t}")
                    for k2 in range(NKC // 2):
                        nc.tensor.matmul(
                            den[0:1, 0, :], ones2[:, 0:2, 0:1],
                            ex[:, 2 * k2:2 * k2 + 2, :],
                            start=(k2 == 0), stop=(k2 == NKC // 2 - 1),
                            perf_mode=DR,
                        )
                    ctxT = ctxp.tile([P, C, 512], f8, tag="ctxT")
                    for u in range(C):
                        nc.vector.tensor_scalar_mul(
                            ctxT[:, u, :], ctx_psums[u][:], CTX_SCL)

                    den_row = small.tile([1, 512], f32, tag="den_row", bufs=2)
                    nc.scalar.copy(den_row[:], den[0:1, 0, :])
                    for s in range(4):
                        # K=1 matmul == transpose of a 128-wide row slice
                        nc.tensor.matmul(
                            den[:, 1, s:s + 1], den_row[0:1, s * P:(s + 1) * P],
                            one_one[:], start=True, stop=True,
                        )
                    recip = small.tile([P, C], f32, tag="recip", bufs=2)
                    nc.vector.reciprocal(recip[:], den[:, 1, 0:4])

                    for s in range(4):  # nq sub-chunks of 128
                        pj = ctx_ps.tile([P, 512], f32, tag="ctx",
                                         name=f"pj_{t}_{s}")
                        for c2 in range(2):
                            nc.tensor.matmul(
                                pj[:],
                                ctxT[:, 2 * c2:2 * c2 + 2, s * P:(s + 1) * P],
                                wp[:, 2 * c2:2 * c2 + 2, :],
                                start=(c2 == 0), stop=(c2 == 1),
                                perf_mode=DR,
                            )
                        r0 = t * 4 + s
                        o = iop.tile([P, U], bf16, tag="o")
                        # o = pj * recip (per-partition) + xq
                        nc.vector.scalar_tensor_tensor(
                            o[:], pj[:], recip[:, s:s + 1], xq_sb[:, r0, :],
                            Mult, Add,
                        )
                        nc.sync.dma_start(
                            out_d[r0 * P:(r0 + 1) * P, :], o[:])

    nc.compile()
    return nc


def _get_nc():
    if "nc" not in _CACHE:
        _CACHE["nc"] = _build_nc()
    return _CACHE["nc"]


def make_in_maps(x, Wq, bq, Wk, bk, Wv, bv, Wp, bp):
    x = np.asarray(x, np.float32)
    Wq = np.asarray(Wq, np.float32)
    bq = np.asarray(bq, np.float32)
    Wk = np.asarray(Wk, np.float32)
    bk = np.asarray(bk, np.float32)
    Wv = np.asarray(Wv, np.float32)
    bv = np.asarray(bv, np.float32)
    Wp = np.asarray(Wp, np.float32)
    bp = np.asarray(bp, np.float32)

    # attn rows sum to 1 => bv contributes bv @ Wp to every output row
    bconst = (bv @ Wp + bp).astype(np.float32)

    import ml_dtypes
    f8 = ml_dtypes.float8_e4m3
    bf16 = ml_dtypes.bfloat16

    def q8(a):
        return np.ascontiguousarray(np.clip(a, -240.0, 240.0)).astype(f8)

    Wq8, Wk8 = q8(Wq * WS), q8(Wk * WS)
    Wv8, Wp8 = q8(Wv * WS), q8(Wp * WS)

    in_maps = []
    for core in range(NCORES):
        b, h = core // 2, core % 2
        xb = np.ascontiguousarray(x[b])                       # [N, U]
        xbT8 = q8(xb.T)                                       # [U, N]
        in_maps.append({
            "xkvT": np.ascontiguousarray(xbT8[:, (1 - h) * NQ:(2 - h) * NQ]),
            "xqT": np.ascontiguousarray(xbT8[:, h * NQ:(h + 1) * NQ]),
            "xq": np.ascontiguousarray(
                xb[h * NQ:(h + 1) * NQ] + bconst[None, :]).astype(bf16),
            "Wq": Wq8, "Wk": Wk8, "Wv": Wv8, "Wp": Wp8,
            "bq": bq * WS, "bk": bk * WS,
        })
    return in_maps


def gather_out(results):
    out = np.empty((B, N, U), np.float32)
    for core in range(NCORES):
        b, h = core // 2, core % 2
        out[b, h * NQ:(h + 1) * NQ] = np.asarray(
            results[core]["out"], dtype=np.float32)
    return out


def kernel(x, Wq, bq, Wk, bk, Wv, bv, Wp, bp):
    from concourse.bass_utils import run_bass_kernel_spmd

    nc = _get_nc()
    in_maps = make_in_maps(x, Wq, bq, Wk, bk, Wv, bv, Wp, bp)
    res = run_bass_kernel_spmd(nc, in_maps, core_ids=list(range(NCORES)))
    return gather_out(res.results)


# revision 29
# speedup vs baseline: 1.0081x; 1.0029x over previous
"""Trainium2 Bass kernel: single-head self-attention with residual.

Reference computation (per batch b):
    q = x @ Wq + bq ; k = x @ Wk + bk ; v = x @ Wv + bv
    scores = q @ k^T / sqrt(U) ; attn = softmax(scores, axis=-1)
    out = x + (attn @ v) @ Wp + bp

Shapes: x [B=4, N=4096, U=512], weights [512, 512], biases [512].

Sharding: 8 cores = 4 batches x 2 sequence halves. Core i owns batch
b = i // 2, Q-rows h = i % 2 (2048 rows). Each core receives its
batch's FULL x (host-side replication plays the role of the K/V
all-gather), so there are no on-device collectives and cores are fully
independent.

Device layout choices:
  - All matmuls run in fp8e4 (TRN E4M3, max +-240) with
    perf_mode=DoubleRow: operands carry a [128, 2, *] AP (stationary
    [128, 2, 128], moving [128, 2, 512]) so each instruction contracts
    256 -- half the instruction count of bf16; measured ~216 ns per MM
    at full clock (~the 2x fp8 peak rate).
  - Scale management keeps every fp8 tensor well below the 240
    overflow: weights are scaled by WS=16 on the host (keeps N(0,1/512)
    entries out of the subnormal range), scores come out 256x, folded
    into the softmax exp scale; exp gets a -SHIFT bias (softmax is
    shift-invariant) so ex <= e^(smax-SHIFT) ~ 45; ctx is written at
    CTX_SCL=1/32. The denominator 'ones' stationary is 8.0 so the
    reciprocal directly absorbs the leftover 16*16*CTX_SCL/256 = 1/8.
  - x arrives pre-transposed from the host (xkvT [U, N]) so K^T / Q^T
    land directly in the [U, seq] layout the TensorEngine wants; no
    on-device transposes at all.
  - Scores are computed transposed (S^T [nk, nq] tiles), exp'd on the
    Scalar engine straight out of PSUM, and consumed as the moving
    operand of the PV matmul -- flash-attention style. The softmax
    denominator is a DoubleRow ones-matmul over the ex tiles.
  - Score PSUM tiles are PAIRED ([128, 2, 512] spanning 2 banks, one
    exp per pair) and the output projection shares the ctx PSUM ring,
    whose natural slot rotation (ctx0-3 -> pj0-3 -> next ctx0-3)
    pipelines the per-tile tail without explicit deferral.
  - HBM pressure: phase B streams only bf16 xq (prefetched once) and
    bf16 outputs; profiling showed fp32 residual/output traffic pushed
    HBM utilization to ~0.9 and co-limited the kernel.
  - bv/bp are folded on the host: attn rows sum to 1, so
    out = xq + (attn @ (x@Wv)) @ Wp + (bv @ Wp + bp).
"""

import numpy as np

B, N, U = 4, 4096, 512
NCORES = 8
NQ = N // 2          # 2048 Q rows per core
P = 128              # partitions
C = U // P           # 4 u-chunks
NKC = N // P         # 32 nk chunks
NKT = N // 512       # 8 nk 512-tiles
NQT = NQ // 512      # 4 nq 512-tiles
NQC = NQ // P        # 16 q-row chunks of 128
SCALE = float(1.0 / np.sqrt(np.float32(U)))
WS = 16.0            # host-side weight scale (keeps W out of fp8 subnormals)
SHIFT = 3.0          # softmax shift: ex = exp(s - SHIFT)
CTX_SCL = 1.0 / 32.0  # ctx psum -> fp8 scale
# exp input: psum = (16q).(16k) = 256*qk  ->  scale = SCALE/256
SCALE_EFF = SCALE / (WS * WS)
# out = pj * recip + xq needs recip = 1/(WS*WS*CTX_SCL*den) = 1/(8*den):
# the denominator 'ones' stationary is DEN_W so reciprocal(DEN_W*den) works.
DEN_W = WS * WS * CTX_SCL

_CACHE = {}


def warm_ps_out(t):
    return t[:]


def _build_nc():
    from concourse import bacc, mybir, tile

    f32 = mybir.dt.float32
    bf16 = mybir.dt.bfloat16
    f8 = mybir.dt.float8e4
    Ident = mybir.ActivationFunctionType.Identity
    Exp = mybir.ActivationFunctionType.Exp
    Mult = mybir.AluOpType.mult
    Add = mybir.AluOpType.add
    DR = mybir.MatmulPerfMode.DoubleRow

    nc = bacc.Bacc("TRN2", target_bir_lowering=False, debug=False, num_devices=NCORES)

    xkvT_d = nc.dram_tensor("xkvT", [U, NQ], f8, kind="ExternalInput")
    xqT_d = nc.dram_tensor("xqT", [U, NQ], f8, kind="ExternalInput")
    xq_d = nc.dram_tensor("xq", [NQ, U], bf16, kind="ExternalInput")
    Wq_d = nc.dram_tensor("Wq", [U, U], f8, kind="ExternalInput")
    Wk_d = nc.dram_tensor("Wk", [U, U], f8, kind="ExternalInput")
    Wv_d = nc.dram_tensor("Wv", [U, U], f8, kind="ExternalInput")
    Wp_d = nc.dram_tensor("Wp", [U, U], f8, kind="ExternalInput")
    bq_d = nc.dram_tensor("bq", [U], f32, kind="ExternalInput")
    bk_d = nc.dram_tensor("bk", [U], f32, kind="ExternalInput")
    out_d = nc.dram_tensor("out", [NQ, U], bf16, kind="ExternalOutput")

    with tile.TileContext(nc) as tc:
        with (
            tc.tile_pool(name="big", bufs=1) as big,
            tc.tile_pool(name="small", bufs=1) as small,
            tc.tile_pool(name="dram", bufs=2, space="DRAM") as dramp,
        ):
            # ---- persistent tensors -------------------------------------
            kT = big.tile([P, C, N], f8, tag="kT")        # 16*k^T  [u, nk]
            qT = big.tile([P, C, NQ], f8, tag="qT")       # 16*q^T  [u, nq]
            v = big.tile([P, NKC, U], f8, tag="v")        # 16*v    [nk, u]
            xq_sb = big.tile([P, NQC, U], bf16, tag="xq_sb")  # residual+bconst

            bq_sb = small.tile([P, C], f32, tag="bq")
            bk_sb = small.tile([P, C], f32, tag="bk")
            nc.sync.dma_start(bq_sb[:], bq_d.ap().rearrange("(c p) -> p c", p=P))
            nc.sync.dma_start(bk_sb[:], bk_d.ap().rearrange("(c p) -> p c", p=P))

            ones2 = small.tile([P, 2, 16], f8, tag="ones2")
            nc.vector.memset(ones2[:], DEN_W)
            negshift = small.tile([P, 1], f32, tag="negshift")
            nc.vector.memset(negshift[:], -SHIFT)
            one_one = small.tile([1, 1], f32, tag="one_one")
            nc.vector.memset(one_one[:], 1.0)

            xkvT_r = xkvT_d.ap().rearrange("(c p) n -> p c n", p=P)
            xqT_r = xqT_d.ap().rearrange("(c p) n -> p c n", p=P)

            # ---- phase A: projections -----------------------------------
            with (
                tc.tile_pool(name="w3", bufs=1) as w3,
                tc.tile_pool(name="stream", bufs=4) as stream,
                tc.tile_pool(name="pa_ps", bufs=6, space="PSUM") as pa_ps,
            ):
                warm = w3.tile([P, 512], bf16, tag="warm")
                nc.vector.memset(warm[:], 0.0)
                wps = pa_ps.tile([P, 512], f32, tag="warm_ps", name="warm_ps",
                                 bufs=1)
                for i in range(16):
                    nc.tensor.matmul(
                        warm_ps_out(wps), warm[:, 0:P], warm[:],
                        start=(i == 0), stop=False,
                    )

                wq = w3.tile([P, C, U], f8, tag="wq")
                wk = w3.tile([P, C, U], f8, tag="wk")
                wv = w3.tile([P, C, U], f8, tag="wv")
                wq_r = Wq_d.ap().rearrange("(c p) n -> p c n", p=P)
                wk_r = Wk_d.ap().rearrange("(c p) n -> p c n", p=P)
                wv_r = Wv_d.ap().rearrange("(c p) n -> p c n", p=P)
                for c in range(C):
                    nc.sync.dma_start(wq[:, c, :], wq_r[:, c, :])

                # qT[u_out, i] = sum_u_in Wq[u_in, u_out] * xqT[u_in, i] (+bq)
                xts_q = []
                for t in range(NQT):
                    xt = stream.tile([P, C, 512], f8, tag="xt",
                                     name=f"xt_q{t}")
                    xts_q.append(xt)
                    for c in range(C):
                        nc.sync.dma_start(
                            xt[:, c, :], xqT_r[:, c, t * 512:(t + 1) * 512])
                    if t == 2:
                        for c in range(C):
                            nc.sync.dma_start(wk[:, c, :], wk_r[:, c, :])
                            nc.sync.dma_start(wv[:, c, :], wv_r[:, c, :])
                    for m in range(C):
                        ps = pa_ps.tile([P, 512], f32, tag="pa")
                        for c2 in range(2):
                            nc.tensor.matmul(
                                ps[:], wq[:, 2 * c2:2 * c2 + 2, m * P:(m + 1) * P],
                                xt[:, 2 * c2:2 * c2 + 2, :],
                                start=(c2 == 0), stop=(c2 == 1),
                                perf_mode=DR,
                            )
                        if m % 2 == 0:
                            nc.scalar.activation(
                                qT[:, m, t * 512:(t + 1) * 512], ps[:], Ident,
                                bias=bq_sb[:, m:m + 1],
                            )
                        else:
                            nc.vector.tensor_scalar_add(
                                qT[:, m, t * 512:(t + 1) * 512], ps[:],
                                bq_sb[:, m:m + 1],
                            )
                    if t in (1, 2):  # keep the PE hot through the DMA wait
                        for i in range(12):
                            nc.tensor.matmul(
                                warm_ps_out(wps), warm[:, 0:P], warm[:],
                                start=False, stop=(t == 2 and i == 11),
                            )
                nc.scalar.copy(warm[:, 0:4], wps[:, 0:4])  # retire warm psum

                # kT like qT; v[j, u] = sum_u_in x^T[u_in, j] * Wv[u_in, u]
                # key tiles 0-3 are the (resident) Q tiles: keys are laid
                # out [own half, other half] -- a permutation softmax is
                # invariant to since kT and v share it. Tiles 4-7 stream
                # the sibling half from xkvT.
                for t0 in range(0, NKT, 2):
                    if t0 < NQT:
                        xts = [xts_q[t0], xts_q[t0 + 1]]
                    else:
                        xts = []
                        for t in (t0, t0 + 1):
                            xt = stream.tile([P, C, 512], f8, tag="xt",
                                             name=f"xt_kv_{t}")
                            for c in range(C):
                                nc.sync.dma_start(
                                    xt[:, c, :],
                                    xkvT_r[:, c, (t - NQT) * 512:
                                           (t - NQT + 1) * 512])
                            xts.append(xt)
                    for t, xt in zip((t0, t0 + 1), xts):
                        for m in range(C):
                            ps = pa_ps.tile([P, 512], f32, tag="pa")
                            for c2 in range(2):
                                nc.tensor.matmul(
                                    ps[:],
                                    wk[:, 2 * c2:2 * c2 + 2, m * P:(m + 1) * P],
                                    xt[:, 2 * c2:2 * c2 + 2, :],
                                    start=(c2 == 0), stop=(c2 == 1),
                                    perf_mode=DR,
                                )
                            nc.scalar.activation(
                                kT[:, m, t * 512:(t + 1) * 512], ps[:], Ident,
                                bias=bk_sb[:, m:m + 1],
                            )
                    for t, xt in zip((t0, t0 + 1), xts):
                        for m in range(4):  # nk sub-chunks of this 512-tile
                            ps = pa_ps.tile([P, 512], f32, tag="pa")
                            for c2 in range(2):
                                nc.tensor.matmul(
                                    ps[:],
                                    xt[:, 2 * c2:2 * c2 + 2, m * P:(m + 1) * P],
                                    wv[:, 2 * c2:2 * c2 + 2, :],
                                    start=(c2 == 0), stop=(c2 == 1),
                                    perf_mode=DR,
                                )
                            nc.vector.tensor_copy(v[:, t * 4 + m, :], ps[:])

            # ---- phase B: attention + projection ------------------------
            with (
                tc.tile_pool(name="wpp", bufs=1) as wpp,
                tc.tile_pool(name="expp", bufs=2) as expp,
                tc.tile_pool(name="ctxp", bufs=2) as ctxp,
                tc.tile_pool(name="io", bufs=3) as iop,
                tc.tile_pool(name="st_ps", bufs=2, space="PSUM") as st_ps,
                tc.tile_pool(name="ctx_ps", bufs=4, space="PSUM") as ctx_ps,
            ):
                wp = wpp.tile([P, C, U], f8, tag="wp")
                nc.sync.dma_start(wp[:], Wp_d.ap().rearrange("(c p) n -> p c n", p=P))
                # residual (+ folded bias) prefetch, bf16, one strided DMA
                nc.sync.dma_start(
                    xq_sb[:], xq_d.ap().rearrange("(c p) u -> p c u", p=P))

                for t in range(NQT):
                    nq_sl = slice(t * 512, (t + 1) * 512)
                    ctx_psums = [
                        ctx_ps.tile([P, 512], f32, tag="ctx", name=f"ctx_{t}_{u}")
                        for u in range(C)
                    ]

                    ex = expp.tile([P, NKC, 512], f8, tag="ex")
                    for kk in range(0, NKC, 4):
                        for nk in range(kk, kk + 4, 2):
                            # paired score tiles: one 2-bank PSUM tile, one
                            # [128, 2, 512] exp per two key chunks
                            st = st_ps.tile([P, 2, 512], f32, tag="st")
                            for j in range(2):
                                for c2 in range(2):
                                    nc.tensor.matmul(
                                        st[:, j, :],
                                        kT[:, 2 * c2:2 * c2 + 2,
                                           (nk + j) * P:(nk + j + 1) * P],
                                        qT[:, 2 * c2:2 * c2 + 2, nq_sl],
                                        start=(c2 == 0), stop=(c2 == 1),
                                        perf_mode=DR,
                                    )
                            nc.scalar.activation(
                                ex[:, nk:nk + 2, :], st[:], Exp,
                                scale=SCALE_EFF, bias=negshift[:])
                        for k2 in range(kk // 2, kk // 2 + 2):
                            for u in range(C):
                                nc.tensor.matmul(
                                    ctx_psums[u][:],
                                    v[:, 2 * k2:2 * k2 + 2, u * P:(u + 1) * P],
                                    ex[:, 2 * k2:2 * k2 + 2, :],
                                    start=(k2 == 0), stop=(k2 == NKC // 2 - 1),
                                    perf_mode=DR,
                                )
                    # softmax denominator: DoubleRow ones-matmul over ex;
                    # den row lives in bank 0 of an st-ring slot, its
                    # transpose (denT) in bank 1 of the same slot.
                    den = st_ps.tile([P, 2, 512], f32, tag="st", name=f"den_{t}")
                    for k2 in range(NKC // 2):
                        nc.tensor.matmul(
                            den[0:1, 0, :], ones2[:, 0:2, 0:1],
                            ex[:, 2 * k2:2 * k2 + 2, :],
                            start=(k2 == 0), stop=(k2 == NKC // 2 - 1),
                            perf_mode=DR,
                        )
                    ctxT = ctxp.tile([P, C, 512], f8, tag="ctxT")
                    for u in range(C):
                        nc.vector.tensor_scalar_mul(
                            ctxT[:, u, :], ctx_psums[u][:], CTX_SCL)

                    den_row = small.tile([1, 512], f32, tag="den_row", bufs=2)
                    nc.scalar.copy(den_row[:], den[0:1, 0, :])
                    for s in range(4):
                        # K=1 matmul == transpose of a 128-wide row slice
                        nc.tensor.matmul(
                            den[:, 1, s:s + 1], den_row[0:1, s * P:(s + 1) * P],
                            one_one[:], start=True, stop=True,
                        )
                    recip = small.tile([P, C], f32, tag="recip", bufs=2)
                    nc.vector.reciprocal(recip[:], den[:, 1, 0:4])

                    for s in range(4):  # nq sub-chunks of 128
                        pj = ctx_ps.tile([P, 512], f32, tag="ctx",
                                         name=f"pj_{t}_{s}")
                        for c2 in range(2):
                            nc.tensor.matmul(
                                pj[:],
                                ctxT[:, 2 * c2:2 * c2 + 2, s * P:(s + 1) * P],
                                wp[:, 2 * c2:2 * c2 + 2, :],
                                start=(c2 == 0), stop=(c2 == 1),
                                perf_mode=DR,
                            )
                        r0 = t * 4 + s
                        o = iop.tile([P, U], bf16, tag="o")
                        # o = pj * recip (per-partition) + xq
                        nc.vector.scalar_tensor_tensor(
                            o[:], pj[:], recip[:, s:s + 1], xq_sb[:, r0, :],
                            Mult, Add,
                        )
                        nc.sync.dma_start(
                            out_d[r0 * P:(r0 + 1) * P, :], o[:])

    nc.compile()
    return nc


def _get_nc():
    if "nc" not in _CACHE:
        _CACHE["nc"] = _build_nc()
    return _CACHE["nc"]


def make_in_maps(x, Wq, bq, Wk, bk, Wv, bv, Wp, bp):
    x = np.asarray(x, np.float32)
    Wq = np.asarray(Wq, np.float32)
    bq = np.asarray(bq, np.float32)
    Wk = np.asarray(Wk, np.float32)
    bk = np.asarray(bk, np.float32)
    Wv = np.asarray(Wv, np.float32)
    bv = np.asarray(bv, np.float32)
    Wp = np.asarray(Wp, np.float32)
    bp = np.asarray(bp, np.float32)

    # attn rows sum to 1 => bv contributes bv @ Wp to every output row
    bconst = (bv @ Wp + bp).astype(np.float32)

    import ml_dtypes
    f8 = ml_dtypes.float8_e4m3
    bf16 = ml_dtypes.bfloat16

    def q8(a):
        return np.ascontiguousarray(np.clip(a, -240.0, 240.0)).astype(f8)

    Wq8, Wk8 = q8(Wq * WS), q8(Wk * WS)
    Wv8, Wp8 = q8(Wv * WS), q8(Wp * WS)

    in_maps = []
    for core in range(NCORES):
        b, h = core // 2, core % 2
        xb = np.ascontiguousarray(x[b])                       # [N, U]
        xbT8 = q8(xb.T)                                       # [U, N]
        in_maps.append({
            "xkvT": np.ascontiguousarray(xbT8[:, (1 - h) * NQ:(2 - h) * NQ]),
            "xqT": np.ascontiguousarray(xbT8[:, h * NQ:(h + 1) * NQ]),
            "xq": np.ascontiguousarray(
                xb[h * NQ:(h + 1) * NQ] + bconst[None, :]).astype(bf16),
            "Wq": Wq8, "Wk": Wk8, "Wv": Wv8, "Wp": Wp8,
            "bq": bq * WS, "bk": bk * WS,
        })
    return in_maps


def gather_out(results):
    out = np.empty((B, N, U), np.float32)
    for core in range(NCORES):
        b, h = core // 2, core % 2
        out[b, h * NQ:(h + 1) * NQ] = np.asarray(
            results[core]["out"], dtype=np.float32)
    return out


def kernel(x, Wq, bq, Wk, bk, Wv, bv, Wp, bp):
    from concourse.bass_utils import run_bass_kernel_spmd

    nc = _get_nc()
    in_maps = make_in_maps(x, Wq, bq, Wk, bk, Wv, bv, Wp, bp)
    res = run_bass_kernel_spmd(nc, in_maps, core_ids=list(range(NCORES)))
    return gather_out(res.results)


# revision 30
# speedup vs baseline: 1.0127x; 1.0046x over previous
"""Trainium2 Bass kernel: single-head self-attention with residual.

Reference computation (per batch b):
    q = x @ Wq + bq ; k = x @ Wk + bk ; v = x @ Wv + bv
    scores = q @ k^T / sqrt(U) ; attn = softmax(scores, axis=-1)
    out = x + (attn @ v) @ Wp + bp

Shapes: x [B=4, N=4096, U=512], weights [512, 512], biases [512].

Sharding: 8 cores = 4 batches x 2 sequence halves. Core i owns batch
b = i // 2, Q-rows h = i % 2 (2048 rows). Each core receives its
batch's FULL x (host-side replication plays the role of the K/V
all-gather), so there are no on-device collectives and cores are fully
independent.

Device layout choices:
  - All matmuls run in fp8e4 (TRN E4M3, max +-240) with
    perf_mode=DoubleRow: operands carry a [128, 2, *] AP (stationary
    [128, 2, 128], moving [128, 2, 512]) so each instruction contracts
    256 -- half the instruction count of bf16; measured ~216 ns per MM
    at full clock (~the 2x fp8 peak rate).
  - Scale management keeps every fp8 tensor well below the 240
    overflow: weights are scaled by WS=16 on the host (keeps N(0,1/512)
    entries out of the subnormal range), scores come out 256x, folded
    into the softmax exp scale; exp gets a -SHIFT bias (softmax is
    shift-invariant) so ex <= e^(smax-SHIFT) ~ 45; ctx is written at
    CTX_SCL=1/32. The denominator 'ones' stationary is 8.0 so the
    reciprocal directly absorbs the leftover 16*16*CTX_SCL/256 = 1/8.
  - x arrives pre-transposed from the host (xkvT [U, N]) so K^T / Q^T
    land directly in the [U, seq] layout the TensorEngine wants; no
    on-device transposes at all.
  - Scores are computed transposed (S^T [nk, nq] tiles), exp'd on the
    Scalar engine straight out of PSUM, and consumed as the moving
    operand of the PV matmul -- flash-attention style. The softmax
    denominator is a DoubleRow ones-matmul over the ex tiles.
  - Score PSUM tiles are PAIRED ([128, 2, 512] spanning 2 banks, one
    exp per pair) and the output projection shares the ctx PSUM ring,
    whose natural slot rotation (ctx0-3 -> pj0-3 -> next ctx0-3)
    pipelines the per-tile tail without explicit deferral.
  - HBM pressure: phase B streams only bf16 xq (prefetched once) and
    bf16 outputs; profiling showed fp32 residual/output traffic pushed
    HBM utilization to ~0.9 and co-limited the kernel.
  - bv/bp are folded on the host: attn rows sum to 1, so
    out = xq + (attn @ (x@Wv)) @ Wp + (bv @ Wp + bp).
"""

import numpy as np

B, N, U = 4, 4096, 512
NCORES = 8
NQ = N // 2          # 2048 Q rows per core
P = 128              # partitions
C = U // P           # 4 u-chunks
NKC = N // P         # 32 nk chunks
NKT = N // 512       # 8 nk 512-tiles
NQT = NQ // 512      # 4 nq 512-tiles
NQC = NQ // P        # 16 q-row chunks of 128
SCALE = float(1.0 / np.sqrt(np.float32(U)))
WS = 16.0            # host-side weight scale (keeps W out of fp8 subnormals)
SHIFT = 3.0          # softmax shift: ex = exp(s - SHIFT)
CTX_SCL = 1.0 / 32.0  # ctx psum -> fp8 scale
# exp input: psum = (16q).(16k) = 256*qk  ->  scale = SCALE/256
SCALE_EFF = SCALE / (WS * WS)
# out = pj * recip + xq needs recip = 1/(WS*WS*CTX_SCL*den) = 1/(8*den):
# the denominator 'ones' stationary is DEN_W so reciprocal(DEN_W*den) works.
DEN_W = WS * WS * CTX_SCL

_CACHE = {}


def warm_ps_out(t):
    return t[:]


def _build_nc():
    from concourse import bacc, mybir, tile

    f32 = mybir.dt.float32
    bf16 = mybir.dt.bfloat16
    f8 = mybir.dt.float8e4
    Ident = mybir.ActivationFunctionType.Identity
    Exp = mybir.ActivationFunctionType.Exp
    Mult = mybir.AluOpType.mult
    Add = mybir.AluOpType.add
    DR = mybir.MatmulPerfMode.DoubleRow

    nc = bacc.Bacc("TRN2", target_bir_lowering=False, debug=False, num_devices=NCORES)

    xkvT_d = nc.dram_tensor("xkvT", [U, NQ], f8, kind="ExternalInput")
    xqT_d = nc.dram_tensor("xqT", [U, NQ], f8, kind="ExternalInput")
    xq_d = nc.dram_tensor("xq", [NQ, U], bf16, kind="ExternalInput")
    Wq_d = nc.dram_tensor("Wq", [U, U], f8, kind="ExternalInput")
    Wk_d = nc.dram_tensor("Wk", [U, U], f8, kind="ExternalInput")
    Wv_d = nc.dram_tensor("Wv", [U, U], f8, kind="ExternalInput")
    Wp_d = nc.dram_tensor("Wp", [U, U], f8, kind="ExternalInput")
    bq_d = nc.dram_tensor("bq", [U], f32, kind="ExternalInput")
    bk_d = nc.dram_tensor("bk", [U], f32, kind="ExternalInput")
    out_d = nc.dram_tensor("out", [NQ, U], bf16, kind="ExternalOutput")

    with tile.TileContext(nc) as tc:
        with (
            tc.tile_pool(name="big", bufs=1) as big,
            tc.tile_pool(name="small", bufs=1) as small,
            tc.tile_pool(name="dram", bufs=2, space="DRAM") as dramp,
            tc.tile_pool(name="w3", bufs=1) as w3,
            tc.tile_pool(name="stream", bufs=4) as stream,
            tc.tile_pool(name="st_ps", bufs=2, space="PSUM") as st_ps,
            tc.tile_pool(name="ctx_ps", bufs=4, space="PSUM") as ctx_ps,
        ):
            # ---- persistent tensors -------------------------------------
            kT = big.tile([P, C, N], f8, tag="kT")        # 16*k^T  [u, nk]
            qT = big.tile([P, C, NQ], f8, tag="qT")       # 16*q^T  [u, nq]
            v = big.tile([P, NKC, U], f8, tag="v")        # 16*v    [nk, u]
            xq_sb = big.tile([P, NQC, U], bf16, tag="xq_sb")  # residual+bconst

            bq_sb = small.tile([P, C], f32, tag="bq")
            bk_sb = small.tile([P, C], f32, tag="bk")
            nc.sync.dma_start(bq_sb[:], bq_d.ap().rearrange("(c p) -> p c", p=P))
            nc.sync.dma_start(bk_sb[:], bk_d.ap().rearrange("(c p) -> p c", p=P))

            ones2 = small.tile([P, 2, 16], f8, tag="ones2")
            nc.vector.memset(ones2[:], DEN_W)
            negshift = small.tile([P, 1], f32, tag="negshift")
            nc.vector.memset(negshift[:], -SHIFT)
            one_one = small.tile([1, 1], f32, tag="one_one")
            nc.vector.memset(one_one[:], 1.0)

            xkvT_r = xkvT_d.ap().rearrange("(c p) n -> p c n", p=P)
            xqT_r = xqT_d.ap().rearrange("(c p) n -> p c n", p=P)

            # ---- phase A: projections -----------------------------------
            if True:
                warm = w3.tile([P, 512], bf16, tag="warm")
                nc.vector.memset(warm[:], 0.0)
                wps = st_ps.tile([P, 512], f32, tag="st", name="warm_ps")
                for i in range(16):
                    nc.tensor.matmul(
                        warm_ps_out(wps), warm[:, 0:P], warm[:],
                        start=(i == 0), stop=False,
                    )

                wq = w3.tile([P, C, U], f8, tag="wq")
                wk = w3.tile([P, C, U], f8, tag="wk")
                wv = w3.tile([P, C, U], f8, tag="wv")
                wq_r = Wq_d.ap().rearrange("(c p) n -> p c n", p=P)
                wk_r = Wk_d.ap().rearrange("(c p) n -> p c n", p=P)
                wv_r = Wv_d.ap().rearrange("(c p) n -> p c n", p=P)
                for c in range(C):
                    nc.sync.dma_start(wq[:, c, :], wq_r[:, c, :])

                # qT[u_out, i] = sum_u_in Wq[u_in, u_out] * xqT[u_in, i] (+bq)
                xts_q = []
                for t in range(NQT):
                    xt = stream.tile([P, C, 512], f8, tag="xt",
                                     name=f"xt_q{t}")
                    xts_q.append(xt)
                    for c in range(C):
                        nc.sync.dma_start(
                            xt[:, c, :], xqT_r[:, c, t * 512:(t + 1) * 512])
                    if t == 2:
                        for c in range(C):
                            nc.sync.dma_start(wk[:, c, :], wk_r[:, c, :])
                            nc.sync.dma_start(wv[:, c, :], wv_r[:, c, :])
                    for m in range(C):
                        ps = ctx_ps.tile([P, 512], f32, tag="ctx")
                        for c2 in range(2):
                            nc.tensor.matmul(
                                ps[:], wq[:, 2 * c2:2 * c2 + 2, m * P:(m + 1) * P],
                                xt[:, 2 * c2:2 * c2 + 2, :],
                                start=(c2 == 0), stop=(c2 == 1),
                                perf_mode=DR,
                            )
                        if m % 2 == 0:
                            nc.scalar.activation(
                                qT[:, m, t * 512:(t + 1) * 512], ps[:], Ident,
                                bias=bq_sb[:, m:m + 1],
                            )
                        else:
                            nc.vector.tensor_scalar_add(
                                qT[:, m, t * 512:(t + 1) * 512], ps[:],
                                bq_sb[:, m:m + 1],
                            )
                    if t in (1, 2):  # keep the PE hot through the DMA wait
                        for i in range(12):
                            nc.tensor.matmul(
                                warm_ps_out(wps), warm[:, 0:P], warm[:],
                                start=False, stop=(t == 2 and i == 11),
                            )
                nc.scalar.copy(warm[:, 0:4], wps[:, 0:4])  # retire warm psum

                # kT like qT; v[j, u] = sum_u_in x^T[u_in, j] * Wv[u_in, u]
                # key tiles 0-3 are the (resident) Q tiles: keys are laid
                # out [own half, other half] -- a permutation softmax is
                # invariant to since kT and v share it. Tiles 4-7 stream
                # the sibling half from xkvT.
                for t0 in range(0, NKT, 2):
                    if t0 < NQT:
                        xts = [xts_q[t0], xts_q[t0 + 1]]
                    else:
                        xts = []
                        for t in (t0, t0 + 1):
                            xt = stream.tile([P, C, 512], f8, tag="xt",
                                             name=f"xt_kv_{t}")
                            for c in range(C):
                                nc.sync.dma_start(
                                    xt[:, c, :],
                                    xkvT_r[:, c, (t - NQT) * 512:
                                           (t - NQT + 1) * 512])
                            xts.append(xt)
                    for t, xt in zip((t0, t0 + 1), xts):
                        for m in range(C):
                            ps = ctx_ps.tile([P, 512], f32, tag="ctx")
                            for c2 in range(2):
                                nc.tensor.matmul(
                                    ps[:],
                                    wk[:, 2 * c2:2 * c2 + 2, m * P:(m + 1) * P],
                                    xt[:, 2 * c2:2 * c2 + 2, :],
                                    start=(c2 == 0), stop=(c2 == 1),
                                    perf_mode=DR,
                                )
                            nc.scalar.activation(
                                kT[:, m, t * 512:(t + 1) * 512], ps[:], Ident,
                                bias=bk_sb[:, m:m + 1],
                            )
                    for t, xt in zip((t0, t0 + 1), xts):
                        for m in range(4):  # nk sub-chunks of this 512-tile
                            ps = ctx_ps.tile([P, 512], f32, tag="ctx")
                            for c2 in range(2):
                                nc.tensor.matmul(
                                    ps[:],
                                    xt[:, 2 * c2:2 * c2 + 2, m * P:(m + 1) * P],
                                    wv[:, 2 * c2:2 * c2 + 2, :],
                                    start=(c2 == 0), stop=(c2 == 1),
                                    perf_mode=DR,
                                )
                            nc.vector.tensor_copy(v[:, t * 4 + m, :], ps[:])

            # ---- phase B: attention + projection ------------------------
            with (
                tc.tile_pool(name="wpp", bufs=1) as wpp,
                tc.tile_pool(name="expp", bufs=2) as expp,
                tc.tile_pool(name="ctxp", bufs=2) as ctxp,
                tc.tile_pool(name="io", bufs=3) as iop,
            ):
                wp = wpp.tile([P, C, U], f8, tag="wp")
                nc.sync.dma_start(wp[:], Wp_d.ap().rearrange("(c p) n -> p c n", p=P))
                # residual (+ folded bias) prefetch, bf16, one strided DMA
                nc.sync.dma_start(
                    xq_sb[:], xq_d.ap().rearrange("(c p) u -> p c u", p=P))

                for t in range(NQT):
                    nq_sl = slice(t * 512, (t + 1) * 512)
                    ctx_psums = [
                        ctx_ps.tile([P, 512], f32, tag="ctx", name=f"ctx_{t}_{u}")
                        for u in range(C)
                    ]

                    ex = expp.tile([P, NKC, 512], f8, tag="ex")
                    for kk in range(0, NKC, 4):
                        for nk in range(kk, kk + 4, 2):
                            # paired score tiles: one 2-bank PSUM tile, one
                            # [128, 2, 512] exp per two key chunks
                            st = st_ps.tile([P, 2, 512], f32, tag="st")
                            for j in range(2):
                                for c2 in range(2):
                                    nc.tensor.matmul(
                                        st[:, j, :],
                                        kT[:, 2 * c2:2 * c2 + 2,
                                           (nk + j) * P:(nk + j + 1) * P],
                                        qT[:, 2 * c2:2 * c2 + 2, nq_sl],
                                        start=(c2 == 0), stop=(c2 == 1),
                                        perf_mode=DR,
                                    )
                            nc.scalar.activation(
                                ex[:, nk:nk + 2, :], st[:], Exp,
                                scale=SCALE_EFF, bias=negshift[:])
                        for k2 in range(kk // 2, kk // 2 + 2):
                            for u in range(C):
                                nc.tensor.matmul(
                                    ctx_psums[u][:],
                                    v[:, 2 * k2:2 * k2 + 2, u * P:(u + 1) * P],
                                    ex[:, 2 * k2:2 * k2 + 2, :],
                                    start=(k2 == 0), stop=(k2 == NKC // 2 - 1),
                                    perf_mode=DR,
                                )
                    # softmax denominator: DoubleRow ones-matmul over ex;
                    # den row lives in bank 0 of an st-ring slot, its
                    # transpose (denT) in bank 1 of the same slot.
                    den = st_ps.tile([P, 2, 512], f32, tag="st", name=f"den_{t}")
                    for k2 in range(NKC // 2):
                        nc.tensor.matmul(
                            den[0:1, 0, :], ones2[:, 0:2, 0:1],
                            ex[:, 2 * k2:2 * k2 + 2, :],
                            start=(k2 == 0), stop=(k2 == NKC // 2 - 1),
                            perf_mode=DR,
                        )
                    ctxT = ctxp.tile([P, C, 512], f8, tag="ctxT")
                    for u in range(C):
                        nc.vector.tensor_scalar_mul(
                            ctxT[:, u, :], ctx_psums[u][:], CTX_SCL)

                    den_row = small.tile([1, 512], f32, tag="den_row", bufs=2)
                    nc.scalar.copy(den_row[:], den[0:1, 0, :])
                    for s in range(4):
                        # K=1 matmul == transpose of a 128-wide row slice
                        nc.tensor.matmul(
                            den[:, 1, s:s + 1], den_row[0:1, s * P:(s + 1) * P],
                            one_one[:], start=True, stop=True,
                        )
                    recip = small.tile([P, C], f32, tag="recip", bufs=2)
                    nc.vector.reciprocal(recip[:], den[:, 1, 0:4])

                    for s in range(4):  # nq sub-chunks of 128
                        pj = ctx_ps.tile([P, 512], f32, tag="ctx",
                                         name=f"pj_{t}_{s}")
                        for c2 in range(2):
                            nc.tensor.matmul(
                                pj[:],
                                ctxT[:, 2 * c2:2 * c2 + 2, s * P:(s + 1) * P],
                                wp[:, 2 * c2:2 * c2 + 2, :],
                                start=(c2 == 0), stop=(c2 == 1),
                                perf_mode=DR,
                            )
                        r0 = t * 4 + s
                        o = iop.tile([P, U], bf16, tag="o")
                        # o = pj * recip (per-partition) + xq
                        nc.vector.scalar_tensor_tensor(
                            o[:], pj[:], recip[:, s:s + 1], xq_sb[:, r0, :],
                            Mult, Add,
                        )
                        nc.sync.dma_start(
                            out_d[r0 * P:(r0 + 1) * P, :], o[:])

    nc.compile()
    return nc


def _get_nc():
    if "nc" not in _CACHE:
        _CACHE["nc"] = _build_nc()
    return _CACHE["nc"]


def make_in_maps(x, Wq, bq, Wk, bk, Wv, bv, Wp, bp):
    x = np.asarray(x, np.float32)
    Wq = np.asarray(Wq, np.float32)
    bq = np.asarray(bq, np.float32)
    Wk = np.asarray(Wk, np.float32)
    bk = np.asarray(bk, np.float32)
    Wv = np.asarray(Wv, np.float32)
    bv = np.asarray(bv, np.float32)
    Wp = np.asarray(Wp, np.float32)
    bp = np.asarray(bp, np.float32)

    # attn rows sum to 1 => bv contributes bv @ Wp to every output row
    bconst = (bv @ Wp + bp).astype(np.float32)

    import ml_dtypes
    f8 = ml_dtypes.float8_e4m3
    bf16 = ml_dtypes.bfloat16

    def q8(a):
        return np.ascontiguousarray(np.clip(a, -240.0, 240.0)).astype(f8)

    Wq8, Wk8 = q8(Wq * WS), q8(Wk * WS)
    Wv8, Wp8 = q8(Wv * WS), q8(Wp * WS)

    in_maps = []
    for core in range(NCORES):
        b, h = core // 2, core % 2
        xb = np.ascontiguousarray(x[b])                       # [N, U]
        xbT8 = q8(xb.T)                                       # [U, N]
        in_maps.append({
            "xkvT": np.ascontiguousarray(xbT8[:, (1 - h) * NQ:(2 - h) * NQ]),
            "xqT": np.ascontiguousarray(xbT8[:, h * NQ:(h + 1) * NQ]),
            "xq": np.ascontiguousarray(
                xb[h * NQ:(h + 1) * NQ] + bconst[None, :]).astype(bf16),
            "Wq": Wq8, "Wk": Wk8, "Wv": Wv8, "Wp": Wp8,
            "bq": bq * WS, "bk": bk * WS,
        })
    return in_maps


def gather_out(results):
    out = np.empty((B, N, U), np.float32)
    for core in range(NCORES):
        b, h = core // 2, core % 2
        out[b, h * NQ:(h + 1) * NQ] = np.asarray(
            results[core]["out"], dtype=np.float32)
    return out


def kernel(x, Wq, bq, Wk, bk, Wv, bv, Wp, bp):
    from concourse.bass_utils import run_bass_kernel_spmd

    nc = _get_nc()
    in_maps = make_in_maps(x, Wq, bq, Wk, bk, Wv, bv, Wp, bp)
    res = run_bass_kernel_spmd(nc, in_maps, core_ids=list(range(NCORES)))
    return gather_out(res.results)


# revision 31
# speedup vs baseline: 1.0248x; 1.0120x over previous
"""Trainium2 Bass kernel: single-head self-attention with residual.

Reference computation (per batch b):
    q = x @ Wq + bq ; k = x @ Wk + bk ; v = x @ Wv + bv
    scores = q @ k^T / sqrt(U) ; attn = softmax(scores, axis=-1)
    out = x + (attn @ v) @ Wp + bp

Shapes: x [B=4, N=4096, U=512], weights [512, 512], biases [512].

Sharding: 8 cores = 4 batches x 2 sequence halves. Core i owns batch
b = i // 2, Q-rows h = i % 2 (2048 rows). Each core receives its
batch's FULL x (host-side replication plays the role of the K/V
all-gather), so there are no on-device collectives and cores are fully
independent.

Device layout choices:
  - All matmuls run in fp8e4 (TRN E4M3, max +-240) with
    perf_mode=DoubleRow: operands carry a [128, 2, *] AP (stationary
    [128, 2, 128], moving [128, 2, 512]) so each instruction contracts
    256 -- half the instruction count of bf16; measured ~216 ns per MM
    at full clock (~the 2x fp8 peak rate).
  - Scale management keeps every fp8 tensor well below the 240
    overflow: weights are scaled by WS=16 on the host (keeps N(0,1/512)
    entries out of the subnormal range), scores come out 256x, folded
    into the softmax exp scale; exp gets a -SHIFT bias (softmax is
    shift-invariant) so ex <= e^(smax-SHIFT) ~ 45; ctx is written at
    CTX_SCL=1/32. The denominator 'ones' stationary is 8.0 so the
    reciprocal directly absorbs the leftover 16*16*CTX_SCL/256 = 1/8.
  - x arrives pre-transposed from the host (xkvT [U, N]) so K^T / Q^T
    land directly in the [U, seq] layout the TensorEngine wants; no
    on-device transposes at all.
  - Scores are computed transposed (S^T [nk, nq] tiles), exp'd on the
    Scalar engine straight out of PSUM, and consumed as the moving
    operand of the PV matmul -- flash-attention style. The softmax
    denominator is a DoubleRow ones-matmul over the ex tiles.
  - Score PSUM tiles are PAIRED ([128, 2, 512] spanning 2 banks, one
    exp per pair) and the output projection shares the ctx PSUM ring,
    whose natural slot rotation (ctx0-3 -> pj0-3 -> next ctx0-3)
    pipelines the per-tile tail without explicit deferral.
  - HBM pressure: phase B streams only bf16 xq (prefetched once) and
    bf16 outputs; profiling showed fp32 residual/output traffic pushed
    HBM utilization to ~0.9 and co-limited the kernel.
  - bv/bp are folded on the host: attn rows sum to 1, so
    out = xq + (attn @ (x@Wv)) @ Wp + (bv @ Wp + bp).
"""

import numpy as np

B, N, U = 4, 4096, 512
NCORES = 8
NQ = N // 2          # 2048 Q rows per core
P = 128              # partitions
C = U // P           # 4 u-chunks
NKC = N // P         # 32 nk chunks
NKT = N // 512       # 8 nk 512-tiles
NQT = NQ // 512      # 4 nq 512-tiles
NQC = NQ // P        # 16 q-row chunks of 128
SCALE = float(1.0 / np.sqrt(np.float32(U)))
WS = 16.0            # host-side weight scale (keeps W out of fp8 subnormals)
SHIFT = 3.0          # softmax shift: ex = exp(s - SHIFT)
CTX_SCL = 1.0 / 32.0  # ctx psum -> fp8 scale
# exp input: psum = (16q).(16k) = 256*qk  ->  scale = SCALE/256
SCALE_EFF = SCALE / (WS * WS)
# out = pj * recip + xq needs recip = 1/(WS*WS*CTX_SCL*den) = 1/(8*den):
# the denominator 'ones' stationary is DEN_W so reciprocal(DEN_W*den) works.
DEN_W = WS * WS * CTX_SCL

_CACHE = {}


def warm_ps_out(t):
    return t[:]


def _build_nc():
    from concourse import bacc, mybir, tile

    f32 = mybir.dt.float32
    bf16 = mybir.dt.bfloat16
    f8 = mybir.dt.float8e4
    Ident = mybir.ActivationFunctionType.Identity
    Exp = mybir.ActivationFunctionType.Exp
    Mult = mybir.AluOpType.mult
    Add = mybir.AluOpType.add
    DR = mybir.MatmulPerfMode.DoubleRow

    nc = bacc.Bacc("TRN2", target_bir_lowering=False, debug=False, num_devices=NCORES)

    xkvT_d = nc.dram_tensor("xkvT", [U, NQ], f8, kind="ExternalInput")
    xqT_d = nc.dram_tensor("xqT", [U, NQ], f8, kind="ExternalInput")
    xq_d = nc.dram_tensor("xq", [NQ, U], bf16, kind="ExternalInput")
    Wq_d = nc.dram_tensor("Wq", [U, U], f8, kind="ExternalInput")
    Wk_d = nc.dram_tensor("Wk", [U, U], f8, kind="ExternalInput")
    Wv_d = nc.dram_tensor("Wv", [U, U], f8, kind="ExternalInput")
    Wp_d = nc.dram_tensor("Wp", [U, U], f8, kind="ExternalInput")
    bq_d = nc.dram_tensor("bq", [U], f32, kind="ExternalInput")
    bk_d = nc.dram_tensor("bk", [U], f32, kind="ExternalInput")
    out_d = nc.dram_tensor("out", [NQ, U], bf16, kind="ExternalOutput")

    with tile.TileContext(nc) as tc:
        with (
            tc.tile_pool(name="big", bufs=1) as big,
            tc.tile_pool(name="small", bufs=1) as small,
            tc.tile_pool(name="dram", bufs=2, space="DRAM") as dramp,
            tc.tile_pool(name="w3", bufs=1) as w3,
            tc.tile_pool(name="stream", bufs=4) as stream,
            tc.tile_pool(name="st_ps", bufs=2, space="PSUM") as st_ps,
            tc.tile_pool(name="ctx_ps", bufs=4, space="PSUM") as ctx_ps,
        ):
            # ---- persistent tensors -------------------------------------
            kT = big.tile([P, C, N], f8, tag="kT")        # 16*k^T  [u, nk]
            qT = big.tile([P, C, NQ], f8, tag="qT")       # 16*q^T  [u, nq]
            v = big.tile([P, NKC, U], f8, tag="v")        # 16*v    [nk, u]
            xq_sb = big.tile([P, NQC, U], bf16, tag="xq_sb")  # residual+bconst

            bq_sb = small.tile([P, C], f32, tag="bq")
            bk_sb = small.tile([P, C], f32, tag="bk")
            nc.sync.dma_start(bq_sb[:], bq_d.ap().rearrange("(c p) -> p c", p=P))
            nc.sync.dma_start(bk_sb[:], bk_d.ap().rearrange("(c p) -> p c", p=P))

            ones2 = small.tile([P, 2, 16], f8, tag="ones2")
            nc.vector.memset(ones2[:], DEN_W)
            negshift = small.tile([P, 1], f32, tag="negshift")
            nc.vector.memset(negshift[:], -SHIFT)
            one_one = small.tile([1, 1], f32, tag="one_one")
            nc.vector.memset(one_one[:], 1.0)

            xkvT_r = xkvT_d.ap().rearrange("(c p) n -> p c n", p=P)
            xqT_r = xqT_d.ap().rearrange("(c p) n -> p c n", p=P)

            # ---- phase A: projections -----------------------------------
            if True:
                warm = w3.tile([P, 512], bf16, tag="warm")
                nc.vector.memset(warm[:], 0.0)
                wps = st_ps.tile([P, 512], f32, tag="st", name="warm_ps")
                for i in range(16):
                    nc.tensor.matmul(
                        warm_ps_out(wps), warm[:, 0:P], warm[:],
                        start=(i == 0), stop=False,
                    )

                wq = w3.tile([P, C, U], f8, tag="wq")
                wk = w3.tile([P, C, U], f8, tag="wk")
                wv = w3.tile([P, C, U], f8, tag="wv")
                wq_r = Wq_d.ap().rearrange("(c p) n -> p c n", p=P)
                wk_r = Wk_d.ap().rearrange("(c p) n -> p c n", p=P)
                wv_r = Wv_d.ap().rearrange("(c p) n -> p c n", p=P)
                for c in range(C):
                    nc.sync.dma_start(wq[:, c, :], wq_r[:, c, :])

                # qT[u_out, i] = sum_u_in Wq[u_in, u_out] * xqT[u_in, i] (+bq)
                xts_q = []
                for t in range(NQT):
                    xt = stream.tile([P, C, 512], f8, tag="xt",
                                     name=f"xt_q{t}")
                    xts_q.append(xt)
                    for c in range(C):
                        nc.sync.dma_start(
                            xt[:, c, :], xqT_r[:, c, t * 512:(t + 1) * 512])
                    if t == 2:
                        for c in range(C):
                            nc.sync.dma_start(wk[:, c, :], wk_r[:, c, :])
                            nc.sync.dma_start(wv[:, c, :], wv_r[:, c, :])
                    if t == 3:
                        # K projection of tile 0 runs during the xt3 DMA
                        # wait (needs only wk + the resident xt_q0 tile)
                        for m in range(C):
                            ps = ctx_ps.tile([P, 512], f32, tag="ctx",
                                             name=f"ps_k0_{m}")
                            for c2 in range(2):
                                nc.tensor.matmul(
                                    ps[:],
                                    wk[:, 2 * c2:2 * c2 + 2, m * P:(m + 1) * P],
                                    xts_q[0][:, 2 * c2:2 * c2 + 2, :],
                                    start=(c2 == 0), stop=(c2 == 1),
                                    perf_mode=DR,
                                )
                            nc.scalar.activation(
                                kT[:, m, 0:512], ps[:], Ident,
                                bias=bk_sb[:, m:m + 1],
                            )
                    for m in range(C):
                        ps = ctx_ps.tile([P, 512], f32, tag="ctx")
                        for c2 in range(2):
                            nc.tensor.matmul(
                                ps[:], wq[:, 2 * c2:2 * c2 + 2, m * P:(m + 1) * P],
                                xt[:, 2 * c2:2 * c2 + 2, :],
                                start=(c2 == 0), stop=(c2 == 1),
                                perf_mode=DR,
                            )
                        if m % 2 == 0:
                            nc.scalar.activation(
                                qT[:, m, t * 512:(t + 1) * 512], ps[:], Ident,
                                bias=bq_sb[:, m:m + 1],
                            )
                        else:
                            nc.vector.tensor_scalar_add(
                                qT[:, m, t * 512:(t + 1) * 512], ps[:],
                                bq_sb[:, m:m + 1],
                            )
                    if t in (1, 2):  # keep the PE hot through the DMA wait
                        for i in range(12):
                            nc.tensor.matmul(
                                warm_ps_out(wps), warm[:, 0:P], warm[:],
                                start=False, stop=(t == 2 and i == 11),
                            )
                nc.scalar.copy(warm[:, 0:4], wps[:, 0:4])  # retire warm psum

                # kT like qT; v[j, u] = sum_u_in x^T[u_in, j] * Wv[u_in, u]
                # key tiles 0-3 are the (resident) Q tiles: keys are laid
                # out [own half, other half] -- a permutation softmax is
                # invariant to since kT and v share it. Tiles 4-7 stream
                # the sibling half from xkvT.
                for t0 in range(0, NKT, 2):
                    if t0 < NQT:
                        xts = [xts_q[t0], xts_q[t0 + 1]]
                    else:
                        xts = []
                        for t in (t0, t0 + 1):
                            xt = stream.tile([P, C, 512], f8, tag="xt",
                                             name=f"xt_kv_{t}")
                            for c in range(C):
                                nc.sync.dma_start(
                                    xt[:, c, :],
                                    xkvT_r[:, c, (t - NQT) * 512:
                                           (t - NQT + 1) * 512])
                            xts.append(xt)
                    for t, xt in zip((t0, t0 + 1), xts):
                        if t == 0:
                            continue  # K(tile 0) hoisted into the Q loop
                        for m in range(C):
                            ps = ctx_ps.tile([P, 512], f32, tag="ctx")
                            for c2 in range(2):
                                nc.tensor.matmul(
                                    ps[:],
                                    wk[:, 2 * c2:2 * c2 + 2, m * P:(m + 1) * P],
                                    xt[:, 2 * c2:2 * c2 + 2, :],
                                    start=(c2 == 0), stop=(c2 == 1),
                                    perf_mode=DR,
                                )
                            nc.scalar.activation(
                                kT[:, m, t * 512:(t + 1) * 512], ps[:], Ident,
                                bias=bk_sb[:, m:m + 1],
                            )
                    for t, xt in zip((t0, t0 + 1), xts):
                        for m in range(4):  # nk sub-chunks of this 512-tile
                            ps = ctx_ps.tile([P, 512], f32, tag="ctx")
                            for c2 in range(2):
                                nc.tensor.matmul(
                                    ps[:],
                                    xt[:, 2 * c2:2 * c2 + 2, m * P:(m + 1) * P],
                                    wv[:, 2 * c2:2 * c2 + 2, :],
                                    start=(c2 == 0), stop=(c2 == 1),
                                    perf_mode=DR,
                                )
                            nc.vector.tensor_copy(v[:, t * 4 + m, :], ps[:])

            # ---- phase B: attention + projection ------------------------
            with (
                tc.tile_pool(name="wpp", bufs=1) as wpp,
                tc.tile_pool(name="expp", bufs=2) as expp,
                tc.tile_pool(name="ctxp", bufs=2) as ctxp,
                tc.tile_pool(name="io", bufs=3) as iop,
            ):
                wp = wpp.tile([P, C, U], f8, tag="wp")
                nc.sync.dma_start(wp[:], Wp_d.ap().rearrange("(c p) n -> p c n", p=P))
                # residual (+ folded bias) prefetch, bf16, one strided DMA
                nc.sync.dma_start(
                    xq_sb[:], xq_d.ap().rearrange("(c p) u -> p c u", p=P))

                for t in range(NQT):
                    nq_sl = slice(t * 512, (t + 1) * 512)
                    ctx_psums = [
                        ctx_ps.tile([P, 512], f32, tag="ctx", name=f"ctx_{t}_{u}")
                        for u in range(C)
                    ]

                    ex = expp.tile([P, NKC, 512], f8, tag="ex")
                    for kk in range(0, NKC, 4):
                        for nk in range(kk, kk + 4, 2):
                            # paired score tiles: one 2-bank PSUM tile, one
                            # [128, 2, 512] exp per two key chunks
                            st = st_ps.tile([P, 2, 512], f32, tag="st")
                            for j in range(2):
                                for c2 in range(2):
                                    nc.tensor.matmul(
                                        st[:, j, :],
                                        kT[:, 2 * c2:2 * c2 + 2,
                                           (nk + j) * P:(nk + j + 1) * P],
                                        qT[:, 2 * c2:2 * c2 + 2, nq_sl],
                                        start=(c2 == 0), stop=(c2 == 1),
                                        perf_mode=DR,
                                    )
                            nc.scalar.activation(
                                ex[:, nk:nk + 2, :], st[:], Exp,
                                scale=SCALE_EFF, bias=negshift[:])
                        for k2 in range(kk // 2, kk // 2 + 2):
                            for u in range(C):
                                nc.tensor.matmul(
                                    ctx_psums[u][:],
                                    v[:, 2 * k2:2 * k2 + 2, u * P:(u + 1) * P],
                                    ex[:, 2 * k2:2 * k2 + 2, :],
                                    start=(k2 == 0), stop=(k2 == NKC // 2 - 1),
                                    perf_mode=DR,
                                )
                    # softmax denominator: DoubleRow ones-matmul over ex;
                    # den row lives in bank 0 of an st-ring slot, its
                    # transpose (denT) in bank 1 of the same slot.
                    den = st_ps.tile([P, 2, 512], f32, tag="st", name=f"den_{t}")
                    for k2 in range(NKC // 2):
                        nc.tensor.matmul(
                            den[0:1, 0, :], ones2[:, 0:2, 0:1],
                            ex[:, 2 * k2:2 * k2 + 2, :],
                            start=(k2 == 0), stop=(k2 == NKC // 2 - 1),
                            perf_mode=DR,
                        )
                    ctxT = ctxp.tile([P, C, 512], f8, tag="ctxT")
                    for u in range(C):
                        nc.vector.tensor_scalar_mul(
                            ctxT[:, u, :], ctx_psums[u][:], CTX_SCL)

                    den_row = small.tile([1, 512], f32, tag="den_row", bufs=2)
                    nc.scalar.copy(den_row[:], den[0:1, 0, :])
                    for s in range(4):
                        # K=1 matmul == transpose of a 128-wide row slice
                        nc.tensor.matmul(
                            den[:, 1, s:s + 1], den_row[0:1, s * P:(s + 1) * P],
                            one_one[:], start=True, stop=True,
                        )
                    recip = small.tile([P, C], f32, tag="recip", bufs=2)
                    nc.vector.reciprocal(recip[:], den[:, 1, 0:4])

                    for s in range(4):  # nq sub-chunks of 128
                        pj = ctx_ps.tile([P, 512], f32, tag="ctx",
                                         name=f"pj_{t}_{s}")
                        for c2 in range(2):
                            nc.tensor.matmul(
                                pj[:],
                                ctxT[:, 2 * c2:2 * c2 + 2, s * P:(s + 1) * P],
                                wp[:, 2 * c2:2 * c2 + 2, :],
                                start=(c2 == 0), stop=(c2 == 1),
                                perf_mode=DR,
                            )
                        r0 = t * 4 + s
                        o = iop.tile([P, U], bf16, tag="o")
                        # o = pj * recip (per-partition) + xq
                        nc.vector.scalar_tensor_tensor(
                            o[:], pj[:], recip[:, s:s + 1], xq_sb[:, r0, :],
                            Mult, Add,
                        )
                        nc.sync.dma_start(
                            out_d[r0 * P:(r0 + 1) * P, :], o[:])

    nc.compile()
    return nc


def _get_nc():
    if "nc" not in _CACHE:
        _CACHE["nc"] = _build_nc()
    return _CACHE["nc"]


def make_in_maps(x, Wq, bq, Wk, bk, Wv, bv, Wp, bp):
    x = np.asarray(x, np.float32)
    Wq = np.asarray(Wq, np.float32)
    bq = np.asarray(bq, np.float32)
    Wk = np.asarray(Wk, np.float32)
    bk = np.asarray(bk, np.float32)
    Wv = np.asarray(Wv, np.float32)
    bv = np.asarray(bv, np.float32)
    Wp = np.asarray(Wp, np.float32)
    bp = np.asarray(bp, np.float32)

    # attn rows sum to 1 => bv contributes bv @ Wp to every output row
    bconst = (bv @ Wp + bp).astype(np.float32)

    import ml_dtypes
    f8 = ml_dtypes.float8_e4m3
    bf16 = ml_dtypes.bfloat16

    def q8(a):
        return np.ascontiguousarray(np.clip(a, -240.0, 240.0)).astype(f8)

    Wq8, Wk8 = q8(Wq * WS), q8(Wk * WS)
    Wv8, Wp8 = q8(Wv * WS), q8(Wp * WS)

    in_maps = []
    for core in range(NCORES):
        b, h = core // 2, core % 2
        xb = np.ascontiguousarray(x[b])                       # [N, U]
        xbT8 = q8(xb.T)                                       # [U, N]
        in_maps.append({
            "xkvT": np.ascontiguousarray(xbT8[:, (1 - h) * NQ:(2 - h) * NQ]),
            "xqT": np.ascontiguousarray(xbT8[:, h * NQ:(h + 1) * NQ]),
            "xq": np.ascontiguousarray(
                xb[h * NQ:(h + 1) * NQ] + bconst[None, :]).astype(bf16),
            "Wq": Wq8, "Wk": Wk8, "Wv": Wv8, "Wp": Wp8,
            "bq": bq * WS, "bk": bk * WS,
        })
    return in_maps


def gather_out(results):
    out = np.empty((B, N, U), np.float32)
    for core in range(NCORES):
        b, h = core // 2, core % 2
        out[b, h * NQ:(h + 1) * NQ] = np.asarray(
            results[core]["out"], dtype=np.float32)
    return out


def kernel(x, Wq, bq, Wk, bk, Wv, bv, Wp, bp):
    from concourse.bass_utils import run_bass_kernel_spmd

    nc = _get_nc()
    in_maps = make_in_maps(x, Wq, bq, Wk, bk, Wv, bv, Wp, bp)
    res = run_bass_kernel_spmd(nc, in_maps, core_ids=list(range(NCORES)))
    return gather_out(res.results)
